# revision 6
# baseline (speedup 1.0000x reference)
"""Trainium2 Bass kernel: 3x3 stride-1 pad-1 Conv2D, NCHW, via 1D Winograd.

Problem: x (32,128,56,56) f32, weight (256,128,3,3) OIHW, bias (256,)
-> out (32,256,56,56) f32.

Strategy: data-parallel over batch N across 8 NeuronCores (4 images per
core), weights/bias replicated. Per core: Winograd F(2,3) along H —
output rows are produced in pairs; the 3 vertical taps collapse into 4
"pos" products shared by both rows of a pair (2x row reuse), cutting PE
streaming cycles 1.5x vs the direct 9-tap implicit GEMM:

  t[pos]    = B^T d        (row combos of the input, DVE fp16)
  m[pos]    = sum_kw Gg[kw,pos]^T @ t[pos](shifted kw)   (PE, PSUM acc.)
  out pair  = A^T m:  o0 = m0+m1+m2,  o1 = m1-m2-m3

The A^T combine is spread across engines so it all hides under the PE:
ACT copies m1,m2 PSUM->SBUF (fp16), GPSIMD forms s=m1+m2, DVE forms
d=m1-m2 and fuses the final adds with the remaining PSUM reads via
scalar_tensor_tensor. Weight transform Gg = G @ W_taps is folded on the
host; bias (zeros in this problem, but handled generally) is added on
the host after gather.

Startup is latency-critical (first DMA data lands ~9.5us after launch):
image 0 is loaded in 16-row block chunks and the weights in per-(c,pos)
tiles so the first matmul group only waits on ~330KB; warm matmuls on a
DVE-memset scratch keep the PE HAM un-throttled until real data lands.
"""

import numpy as np

import concourse.bass as bass
import concourse.mybir as mybir
import concourse.tile as tile
from concourse import bacc
from concourse.bass_utils import run_bass_kernel_spmd

N_CORES = 8
N_FULL = 32
N_PER_CORE = N_FULL // N_CORES  # 4
CIN = 128
COUT = 256
H = W = 56
HP = WP = 58  # padded spatial
NPAIR = H // 2  # 28 row-pairs per image
QB = 7  # row-pairs per block
NB = NPAIR // QB  # 4 blocks per image
NF = QB * W  # 392 matmul free dim (pairs x width)
ROWS = 2 * QB + 2  # 16 padded input rows per block (14 + 2 halo)
F32 = mybir.dt.float32
F16 = mybir.dt.float16

# Module-level knobs for the dev harness (test.py). The grading harness
# just calls kernel(**inputs) and gets the default (no-trace) path.
TRACE = False
LAST_RESULT = None

_prog = None


def _build_program():
    nc = bacc.Bacc("TRN2", target_bir_lowering=False, debug=False)
    x_d = nc.declare_dram_parameter("x", [N_PER_CORE, CIN, HP * WP], F16, isOutput=False)
    # wt[ci, ((c*4 + pos)*3 + kw)*128 + co2] = Gg, host-transformed
    w_d = nc.declare_dram_parameter("wt", [CIN, 24 * 128], F16, isOutput=False)
    out_d = nc.declare_dram_parameter(
        "out", [N_PER_CORE, 2, 128, H * W], F16, isOutput=True
    )

    AluOp = mybir.AluOpType
    ActFn = mybir.ActivationFunctionType

    # A^T combine pos order: m1, m2 first so their PSUM banks close (and
    # the drain chain starts) while the PE still streams m0, m3.
    POS_ORDER = (1, 2, 0, 3)
    # Input-transform row combos per pos: t[pos] = d[a0] op d[a1]
    T_DEFS = (
        (0, 2, AluOp.subtract),  # t0 = d0 - d2
        (1, 2, AluOp.add),       # t1 = d1 + d2
        (2, 1, AluOp.subtract),  # t2 = d2 - d1
        (1, 3, AluOp.subtract),  # t3 = d1 - d3
    )

    with tile.TileContext(nc) as tc:
        with (
            tc.tile_pool(name="const", bufs=1) as const_pool,
            tc.tile_pool(name="xb0", bufs=4) as xb_pool,
            tc.tile_pool(name="xin", bufs=2) as x_pool,
            tc.tile_pool(name="tb0", bufs=3) as tb_pool,
            tc.tile_pool(name="tin", bufs=2) as t_pool,
            tc.tile_pool(name="mc", bufs=3) as mc_pool,
            tc.tile_pool(name="sd", bufs=3) as sd_pool,
            tc.tile_pool(name="outp", bufs=4) as out_pool,
            tc.tile_pool(name="psum", bufs=8, space="PSUM") as psum_pool,
        ):
            # Weights as per-(c,pos) tiles so each matmul group's dependency
            # is its own 3*128-column slice, not the whole weight array.
            w_sbs = {}
            for c in range(2):
                for pos in range(4):
                    w_cp = const_pool.tile([CIN, 3 * 128], F16, tag=f"w{c}p{pos}")
                    w_sbs[(c, pos)] = w_cp

            def load_w(c, pos):
                base = (c * 4 + pos) * 3 * 128
                halfw = 3 * 128 // 2
                for eng, lo, hi in ((nc.sync, 0, halfw), (nc.scalar, halfw, 3 * 128)):
                    eng.dma_start(
                        out=w_sbs[(c, pos)][:, lo:hi], in_=w_d[:, base + lo : base + hi]
                    )

            x_view = x_d[:].rearrange("n p (h w) -> n p h w", w=WP)
            xb_tiles = {}
            x_tiles = {}

            def load_block0(b):
                # Padded input rows [14b, 14b+16) of image 0.
                x_c = xb_pool.tile([CIN, ROWS, WP], F16)
                r0 = b * 2 * QB
                half = ROWS // 2
                for eng, lo, hi in ((nc.sync, 0, half), (nc.scalar, half, ROWS)):
                    eng.dma_start(
                        out=x_c[:, lo:hi, :], in_=x_view[0][:, r0 + lo : r0 + hi, :]
                    )
                xb_tiles[b] = x_c

            def load_image(i):
                x_c = x_pool.tile([CIN, HP, WP], F16)
                half = HP // 2
                for eng, lo, hi in ((nc.sync, 0, half), (nc.scalar, half, HP)):
                    eng.dma_start(out=x_c[:, lo:hi, :], in_=x_view[i][:, lo:hi, :])
                x_tiles[i] = x_c

            # Warmup scratch: DVE memset is fast (~0.2us) and DVE is idle at
            # startup, so warm matmuls begin right at the engine barrier.
            scratch = const_pool.tile([128, NF], F16)
            nc.vector.memset(scratch[:], 0.0)
            warm_ps = psum_pool.tile([128, NF], F32, tag="ps")
            NWARM = 12
            for wi in range(NWARM):
                nc.tensor.matmul(
                    warm_ps[:], lhsT=scratch[:, :128], rhs=scratch[:],
                    start=(wi == 0), stop=(wi == NWARM - 1), skip_group_check=True,
                )

            # Emission order = DMA queue order, sequenced by need time.
            load_block0(0)
            load_w(0, 1)
            load_w(0, 2)
            load_block0(1)
            load_w(0, 0)
            load_w(0, 3)
            load_w(1, 1)
            load_block0(2)
            load_w(1, 2)
            load_w(1, 0)
            load_w(1, 3)
            load_block0(3)
            load_image(1)

            t_tiles = {}
            tb_tiles = {}

            def transform_block0(b):
                xt = xb_tiles[b]
                t_t = tb_pool.tile([CIN, 4, QB, WP], F16)
                E = 2 * QB - 1
                for pos in POS_ORDER:
                    a0, a1, op = T_DEFS[pos]
                    nc.vector.tensor_tensor(
                        t_t[:, pos],
                        xt[:, a0 : a0 + E : 2, :],
                        xt[:, a1 : a1 + E : 2, :],
                        op,
                    )
                tb_tiles[b] = t_t

            def transform_image(i):
                # t[pos] = B^T d over all 28 row pairs at once: even middle
                # dim (28) keeps the DVE 2x perf mode eligible.
                xt = x_tiles[i]
                t_t = t_pool.tile([CIN, 4, NPAIR, WP], F16)
                E = 2 * NPAIR - 1
                for pos in POS_ORDER:
                    a0, a1, op = T_DEFS[pos]
                    nc.vector.tensor_tensor(
                        t_t[:, pos],
                        xt[:, a0 : a0 + E : 2, :],
                        xt[:, a1 : a1 + E : 2, :],
                        op,
                    )
                t_tiles[i] = t_t

            store_ctr = [0]

            def compute_block(i, b, split_drain=False):
                if i == 0:
                    t_t, q_off = tb_tiles[b], 0
                else:
                    t_t, q_off = t_tiles[i], b * QB
                for c in range(2):
                    ms = {}
                    for pos in POS_ORDER:
                        ps = psum_pool.tile([128, NF], F32, tag="ps")
                        ps_v = ps[:].rearrange("p (q w) -> p q w", w=W)
                        for kw in range(3):
                            nc.tensor.matmul(
                                ps_v,
                                lhsT=w_sbs[(c, pos)][:, kw * 128 : (kw + 1) * 128],
                                rhs=t_t[:, pos, q_off : q_off + QB, kw : kw + W],
                                start=(kw == 0), stop=(kw == 2),
                            )
                        ms[pos] = ps
                    # Drain: o0 = m0 + (m1+m2), o1 = (-m3) + (m1-m2)
                    out_t = out_pool.tile([128, QB, 2, W], F16)
                    qsplits = ((0, 4), (4, QB)) if split_drain else ((0, QB),)
                    for q0, q1 in qsplits:
                        sl = slice(q0 * W, q1 * W)
                        mc = mc_pool.tile([128, 2, NF], F16)
                        nc.scalar.activation(mc[:, 0, sl], ms[1][:, sl], ActFn.Copy)
                        nc.scalar.activation(mc[:, 1, sl], ms[2][:, sl], ActFn.Copy)
                        s_t = sd_pool.tile([128, NF], F16, tag="s")
                        d_t = sd_pool.tile([128, NF], F16, tag="d")
                        nc.gpsimd.tensor_tensor(
                            s_t[:, sl], mc[:, 0, sl], mc[:, 1, sl], AluOp.add
                        )
                        nc.vector.tensor_tensor(
                            d_t[:, sl], mc[:, 0, sl], mc[:, 1, sl], AluOp.subtract
                        )
                        for j, m_ps, sd_t, op0, scl in (
                            (0, ms[0], s_t, AluOp.bypass, 0.0),
                            (1, ms[3], d_t, AluOp.mult, -1.0),
                        ):
                            nc.vector.scalar_tensor_tensor(
                                out_t[:, q0:q1, j, :],
                                m_ps[:].rearrange("p (q w) -> p q w", w=W)[:, q0:q1],
                                scl,
                                sd_t[:].rearrange("p (q w) -> p q w", w=W)[:, q0:q1],
                                op0,
                                AluOp.add,
                            )
                        lo = (b * QB + q0) * 2 * W
                        store_eng = nc.sync if store_ctr[0] % 2 == 0 else nc.scalar
                        store_ctr[0] += 1
                        store_eng.dma_start(
                            out=out_d[i, c][:, lo : lo + (q1 - q0) * 2 * W],
                            in_=out_t[:, q0:q1].rearrange("p q j w -> p (q j w)"),
                        )

            for b in range(NB):
                transform_block0(b)

            for i in range(N_PER_CORE):
                if i + 2 < N_PER_CORE:
                    load_image(i + 2)
                for b in range(NB):
                    last = i == N_PER_CORE - 1 and b == NB - 1
                    if b == 1 and i + 1 < N_PER_CORE:
                        transform_image(i + 1)
                    compute_block(i, b, split_drain=last)
                if i > 0:
                    del x_tiles[i], t_tiles[i]
    nc.compile()
    return nc


# F(2,3) weight transform matrix (applied over the kh axis).
_G = np.array(
    [[1.0, 0.0, 0.0], [0.5, 0.5, 0.5], [0.5, -0.5, 0.5], [0.0, 0.0, 1.0]],
    dtype=np.float64,
)


def kernel(x: np.ndarray, weight: np.ndarray, bias: np.ndarray) -> np.ndarray:
    global _prog, LAST_RESULT
    x = np.ascontiguousarray(x, dtype=np.float32)
    weight = np.ascontiguousarray(weight, dtype=np.float32)
    bias = np.ascontiguousarray(bias, dtype=np.float32)

    # Host-side prep: pad spatial dims, shard batch, Winograd-transform the
    # weights over kh: Gg[pos][co,ci,kw] = sum_kh G[pos,kh] W[co,ci,kh,kw].
    x_pad = np.zeros((N_FULL, CIN, HP, WP), dtype=np.float16)
    x_pad[:, :, 1:-1, 1:-1] = x
    x_pad = x_pad.reshape(N_FULL, CIN, HP * WP)

    u = np.einsum("ph,oihw->oipw", _G, weight.astype(np.float64))
    # wt[ci, ((c*4 + pos)*3 + kw)*128 + co2]
    wt = np.ascontiguousarray(
        u.reshape(2, 128, CIN, 4, 3).transpose(2, 0, 3, 4, 1).reshape(CIN, 24 * 128)
    ).astype(np.float16)

    if _prog is None:
        _prog = _build_program()

    in_maps = [
        {
            "x": np.ascontiguousarray(x_pad[i * N_PER_CORE : (i + 1) * N_PER_CORE]),
            "wt": wt,
        }
        for i in range(N_CORES)
    ]
    res = run_bass_kernel_spmd(_prog, in_maps, list(range(N_CORES)), trace=TRACE)
    LAST_RESULT = res
    out = np.concatenate([r["out"] for r in res.results], axis=0)
    out = out.astype(np.float32).reshape(N_FULL, COUT, H, W)
    if bias.any():
        out += bias[None, :, None, None]
    return out
